# revision 47
# baseline (speedup 1.0000x reference)
"""Diagonalizable linear plant (modal state-space scan) on 8 Trainium2 cores.

y[b,t] = Cz @ z[b,t-1] + D @ u[b,t],  z[b,t] = lam * z[b,t-1] + Bz @ u[b,t]
with z[b,-1] = z0[b] = x0[b] @ Q, Bz = Q^T Bmat, Cz = C Q.

Sharding: data-parallel over batch (16 batches -> 2 per core).

Block-8 formulation (the DVE scan instruction runs at ~2.45ns/column,
so the time axis is decimated 8x before it reaches the scan; everything
else is full 128x128xN=512 bf16 matmuls, fp32 PSUM):
  host packs u as uT8[(i*32+u), k] = u[8k+i, u]        (256 rows = 2 K-groups)
  PE   V_h = W2^T @ U          W2[(i,u),n] = lam_n^(7-i) Bz[n,u]
  DVE  zB = scan(lam^8, V)     block-boundary states z_{8k+7}
  PE   Y_g = WC^T @ zBprev + WU^T @ U     (g indexes (j,y) output groups)
       WC[n,(j,y)] = lam_n^j Cz[y,n]
       WU[(i,u),(j,y)] = (Cz lam^(j-1-i) Bz)[y,u] for i<j, D[y,u] for i=j, else 0
  host unpacks yT8[(32j+y), k] -> y[8k+j, y]

v2 scheduling: lam^8/z0 ride a tiny f32 param on the first fast-queue DMA
(no bf16 hi/lo recovery), W2 is split across both HWDGE queues ahead of
the U data so the first V matmul fires ~3us earlier, the Vector engine
runs nothing but the 8 scans back-to-back, and a stream of short dummy
matmuls keeps the PE pstate ramp warm until real work lands.
"""

import numpy as np

B, T, NX, NU, NY = 16, 8192, 256, 32, 32
NCORES = 8
BPC = B // NCORES   # batches per core
MB = 8              # time-block folded into matmul K
KCOL = T // MB      # block columns per batch (1024)
L = 512             # block-columns per chunk
NCHUNK = KCOL // L  # chunks per batch (2)
NWARM = 20          # PE warmup matmuls bridging the DMA fill window

_PROG = None  # built Bass program, cached across kernel() calls


def _patch_cost_model_scan():
    """The tile scheduler's cost model prices DVE ops at 1.04ns/elem, but
    the TensorTensorScan instruction measures ~2.45ns/column on hardware.
    The scans dominate this kernel's Vector load, and the mispricing makes
    the scheduler misplace the whole Y phase. Reprice DVE near the
    measured scan rate so the model's critical path matches hardware."""
    import concourse.hw_specs as hs
    import concourse.mybir as mybir

    if getattr(hs, "_dve_cycle_patched", False):
        return
    ct = dict(hs.TRN2Spec.CYCLE_T)
    ct[mybir.EngineType.DVE] = 1e9 / 0.5e9
    hs.TRN2Spec.CYCLE_T = ct
    hs._dve_cycle_patched = True


def _patch_sim_timed_nop():
    """The scheduling-pass CoreSim rejects the timed NOP (ISA opcode 164)
    used to gate late DMA enqueues. It has no simulator-visible side
    effects — treat it as a plain nop."""
    import concourse.bass_interp as bi

    if getattr(bi, "_timed_nop_patched", False):
        return
    orig = bi._visit_InstISA

    def _visit(isa_mod, instruction, core_sim):
        if instruction.isa_opcode == 164:  # NEURON_ISA_TPB_OPCODE_NOP
            return
        return orig(isa_mod, instruction, core_sim)

    bi._visit_InstISA = _visit
    bi._timed_nop_patched = True


def _patch_tile_drain():
    """walrus codegen in this container rejects >1 sync wait on one SP
    TPB_CTRL instruction (terminal TileContext drain / NoOp). Split the
    drain's waits across preceding SP nops carrying one wait each."""
    import concourse.tile as tile
    import concourse.mybir as mybir
    from concourse.vector_clock import ScopedClock

    if getattr(tile.TileContext, "_drain_patched", False):
        return

    def _drain_and_barrier(self, tick_clock, wait_clock):
        nc = self.nc
        scratch = nc.sync.nop()
        wait_clock.add_sem_waits(
            scratch.ins, ScopedClock({None: tick_clock.global_clock})
        )
        si = scratch.ins.sync_info
        waits = list(si.on_wait) if si is not None else []
        scratch.ins.sync_info = mybir.SyncInfo(on_wait=waits[:1], on_update=[])
        # spread the final sem waits across all engines: a serial ladder of
        # ~20 nops on sync alone costs >1us at the very end of the kernel
        engs = [nc.tensor, nc.vector, nc.scalar, nc.gpsimd, nc.sync]
        for k, w in enumerate(waits[1:]):
            n2 = engs[k % len(engs)].nop()
            n2.ins.sync_info = mybir.SyncInfo(on_wait=[w], on_update=[])
        nc.sync.drain()
        # the wait ladder above already proves every semaphore reached its
        # final value (all DMAs + engine work retired); a sequencer-level
        # barrier is enough to order the clear, and skips the per-engine
        # InstDrain ping-pong (~1us at the very end of the kernel)
        nc.all_engine_barrier(sem_only=True)
        assert self.sems is not None
        popped = nc._tile_sem_poison_stack.pop()
        assert popped is self._sem_poison
        nc.clear_and_free_semaphores(list(self.sems.allocated().values()))

    tile.TileContext._drain_and_barrier = _drain_and_barrier
    tile.TileContext._drain_patched = True


def _split_multi_waits(nc, mybir):
    """This container's walrus codegen accepts at most ONE sync wait per
    instruction. Hoist extra waits into standalone EventSemaphore nops on
    the same engine, placed immediately before the instruction."""
    ctr = [0]

    def fresh(engine, wait):
        ctr[0] += 1
        ev = mybir.InstEventSemaphore(name=f"I-wsplit-{ctr[0]}", ins=[], outs=[])
        ev.engine = engine
        ev.sync_info = mybir.SyncInfo(on_wait=[wait], on_update=[])
        nc.register_instruction(ev)
        return ev

    for fn in nc.m.functions:
        for bb in fn.blocks:
            out = []
            changed = False
            for inst in bb.instructions:
                si = inst.sync_info
                waits = list(si.on_wait) if si is not None else []
                if len(waits) > 1:
                    changed = True
                    for w in waits[:-1]:
                        out.append(fresh(inst.engine, w))
                    inst.sync_info = mybir.SyncInfo(
                        on_wait=[waits[-1]], on_update=list(si.on_update)
                    )
                out.append(inst)
            if changed:
                bb.instructions = out


def build_program():
    import concourse.bass as bass
    import concourse.tile as tile
    import concourse.mybir as mybir
    from contextlib import ExitStack

    _patch_cost_model_scan()
    _patch_sim_timed_nop()
    _patch_tile_drain()
    f32 = mybir.dt.float32
    bf = mybir.dt.bfloat16

    nc = bass.Bass()
    # uH2[ch, g, row, k]: 2 KB rows [b0 | b1] per chunk-half
    uH2 = nc.declare_dram_parameter("uH2", [2, 2, 128, 2 * L], bf, isOutput=False)
    # pzF: col 0:2 = lam^8 f32 (h0, h1); col 2+2b+h = z0 stream (b, h)
    pzF = nc.declare_dram_parameter("pzF", [128, 8], f32, isOutput=False)
    # wAll: W2 [g0h0,g1h0,g0h1,g1h1] | WC [h0g0,h0g1,h1g0,h1g1] | WU00,WU01,WU11
    wAll = nc.declare_dram_parameter("wAll", [128, 11 * 128], bf, isOutput=False)
    yT8s = nc.declare_dram_parameter("yT8s", [BPC, 2, 256, L], bf, isOutput=True)

    with ExitStack() as ctx:
        tc = ctx.enter_context(tile.TileContext(nc))
        const = ctx.enter_context(tc.tile_pool(name="const", bufs=1))
        vps = ctx.enter_context(tc.tile_pool(name="vps", bufs=2, space="PSUM"))
        yps = ctx.enter_context(tc.tile_pool(name="yps", bufs=2, space="PSUM"))
        zpool = ctx.enter_context(tc.tile_pool(name="z", bufs=6))
        yout = ctx.enter_context(tc.tile_pool(name="yo", bufs=4))

        # PE pstate warm-up: short dummy matmuls with no DMA deps keep the
        # Tensor engine continuously busy (the clock ramp needs ~3us of
        # uninterrupted execution to reach 2.4GHz) until real work lands.
        # Emitted FIRST: the tile scheduler keeps per-engine emission order,
        # so the dummy memset must precede everything else on vector.
        dummy = const.tile([128, 256], bf)
        nc.vector.memset(dummy[:], 0.0)
        WP = vps.tile([128, L], f32, name="WP", tag="V0")
        for _ in range(NWARM):
            nc.tensor.matmul(WP[:, 0:256], lhsT=dummy[:, 0:128],
                             rhs=dummy[:], start=True, stop=True)

        # --- DMA plan -----------------------------------------------------
        # Each queue sustains ~110-220 GB/s and throughput collapses for
        # descriptor rows under ~1KB, so: pzF (32B rows) rides the SWDGE
        # queue head where it only costs ~0.3us of lead time, W2 moves as a
        # single 1KB-row transfer, and the unit-0 critical set (W2 / U b0 g0
        # / U b0 g1) is spread across all THREE queues to land in parallel.
        # Transfers queued on the same DMA queue drain CONCURRENTLY (their
        # descriptors interleave), so a late-needed transfer steals
        # bandwidth from a critical one the moment its desc-gen runs. The
        # ch1 transfers are therefore gated behind timed nops on their
        # issuing engines so the ch0+W2 critical set gets the full
        # bandwidth of both HWDGE queues first.
        # sync  (HWDGE): U b0 g0, U b1 g1, U b1 g0, [gate] U ch1 g0;
        #                then g0 output stores and the tail g1 descs.
        # scalar(HWDGE): W2, U b0 g1, [gate] U ch1 g1; Y casts + g0 descs.
        # pool  (SWDGE): pzF, WU, WC; later the g1 output stores.
        pzt = const.tile([128, 8], f32)
        nc.gpsimd.dma_start(pzt[:], pzF[:, :])
        Wt = const.tile([128, 1408], bf)
        nc.scalar.dma_start(Wt[:, 0:512], wAll[:, 0:512])
        # UG[g]: [128, 2048] = [b0ch0 | b1ch0 | b0ch1 | b1ch1].
        # ch0 moves as one 2KB-row transfer per group (b0+b1 together):
        # descriptor rows under 2KB cost queue throughput, and the whole
        # input set is bandwidth-conserved anyway — landing b1 with b0
        # removes the scan3/4 arrival jitter entirely.
        UG = [const.tile([128, 4 * L], bf, name=f"UG{g}") for g in range(2)]
        nc.sync.dma_start(UG[0][:, 0 : 2 * L], uH2[0, 0, :, :])
        nc.scalar.dma_start(UG[1][:, 0 : 2 * L], uH2[0, 1, :, :])
        nc.gpsimd.nop(cycle_cnt=3600)
        nc.gpsimd.dma_start(Wt[:, 1024:1408], wAll[:, 1024:1408])
        nc.gpsimd.dma_start(Wt[:, 512:1024], wAll[:, 512:1024])
        nc.sync.nop(cycle_cnt=2800)
        nc.sync.dma_start(UG[0][:, 2 * L : 4 * L], uH2[1, 0, :, :])
        nc.scalar.nop(cycle_cnt=2000)
        nc.scalar.dma_start(UG[1][:, 2 * L : 4 * L], uH2[1, 1, :, :])

        lam8 = pzt[:, 0:2]
        z0t = pzt[:, 2:6]

        # lam^8 broadcast, built on DVE before any scan can start (vector is
        # otherwise idle during the DMA fill window; scans own it afterward)
        lam_bc = const.tile([128, 3 * L], f32)
        ones = lam_bc[:, 2 * L : 3 * L]
        nc.vector.memset(ones, 1.0)
        for h in range(2):
            nc.vector.tensor_scalar_mul(
                lam_bc[:, h * L : (h + 1) * L], ones, lam8[:, h : h + 1]
            )

        def wblk(i):
            return Wt[:, 128 * i : 128 * (i + 1)]

        W2 = [[wblk(0), wblk(2)], [wblk(1), wblk(3)]]          # [g][h]
        WC = [[wblk(4), wblk(5)], [wblk(6), wblk(7)]]          # [h][g]
        WU00 = wblk(8)
        WU01 = wblk(9)
        WU11 = wblk(10)                                        # WU[1][0] == 0

        mult = mybir.AluOpType.mult
        add = mybir.AluOpType.add

        # prev_z[b] = (Z tiles, end column) — the carry column for chaining
        prev_z = [[None, None] for _ in range(BPC)]
        prev_end = [0 for _ in range(BPC)]

        def emit_vscan(c, b, off, w):
            sl = slice((2 * c + b) * L + off, (2 * c + b) * L + off + w)
            U = [UG[0][:, sl], UG[1][:, sl]]
            zext = [None, None]
            for h in range(2):
                V = vps.tile([128, L], f32, name=f"V{h}_{b}_{c}_{off}",
                             tag=f"V{h}")
                Z = zpool.tile([128, L + 1], bf, name=f"Z{h}_{b}_{c}_{off}",
                               tag=f"Z{h}")
                carry = (z0t[:, 2 * b + h : 2 * b + h + 1]
                         if c == 0 and off == 0
                         else prev_z[b][h][:, prev_end[b] : prev_end[b] + 1])
                nc.tensor.matmul(V[:, 0:w], lhsT=W2[0][h], rhs=U[0],
                                 start=True, stop=False)
                nc.tensor.matmul(V[:, 0:w], lhsT=W2[1][h], rhs=U[1],
                                 start=False, stop=True)
                nc.vector.tensor_tensor_scan(
                    Z[:, 1 : w + 1], lam_bc[:, h * L : h * L + w], V[:, 0:w],
                    carry, mult, add,
                )
                nc.gpsimd.tensor_copy(Z[:, 0:1], carry)
                zext[h] = Z
            prev_z[b] = zext
            prev_end[b] = w
            return U, zext

        # The Y phase is emitted in two waves so the in-order PE queue can
        # frontload all scan-independent work: WU stages (inputs only) go
        # out one unit behind the V matmuls, WC stages (scan consumers)
        # two units behind. Only the last unit's two WC matmuls trail the
        # final scan.
        def emit_wu(c, b, off, w, U, zext, tail_idx=None):
            last = tail_idx == 0
            gorder = [1, 0] if last else [0, 1]
            Yt = {}
            if not last:
                # one [128, 2L] accumulator per unit: both groups cast in
                # ONE scalar op, halving the cast-throughput pressure
                Ypair = yps.tile([128, 2 * L], f32, name=f"Y_{b}_{c}_{off}",
                                 tag="Y")
                Yt["pair"] = Ypair
                for g in gorder:
                    Yt[g] = Ypair[:, g * L : (g + 1) * L]
            else:
                # the final unit's accumulators live in the V-ring banks
                # (dead once its scans retire): its matmuls never wait on
                # the cast-limited yps ring recycling
                for g in gorder:
                    Yt[g] = vps.tile([128, L], f32, name=f"Y{g}_{b}_{c}_{off}",
                                     tag=f"V{1 - g}")
            stages = [(1, WU01, U[0]), (0, WU00, U[0]), (1, WU11, U[1])]
            first = {0: True, 1: True}
            for g, lhsT, rhs in stages:
                nc.tensor.matmul(Yt[g][:, 0:w], lhsT=lhsT, rhs=rhs,
                                 start=first[g], stop=False)
                first[g] = False
            return Yt

        def emit_wc(c, b, off, w, U, zext, Yt, tail_idx=None):
            last = tail_idx == 0
            gorder = [1, 0] if (last or tail_idx == 1) else [0, 1]
            stages = [(gorder[0], 0), (gorder[1], 0), (gorder[0], 1),
                      (gorder[1], 1)]
            for g, h in stages:
                nc.tensor.matmul(Yt[g][:, 0:w], lhsT=WC[h][g],
                                 rhs=zext[h][:, 0:w],
                                 start=False, stop=(h == 1))
            if tail_idx is None:
                Ysb = yout.tile([128, 2 * L], bf, name=f"Ysb_{b}_{c}_{off}",
                                tag="Ysb")
                nc.scalar.copy(Ysb[:], Yt["pair"][:])
                for g in range(2):
                    dst = yT8s[b, c, 128 * g : 128 * (g + 1), off : off + w]
                    oeng = nc.sync if g == 0 else nc.gpsimd
                    oeng.dma_start(dst, Ysb[:, g * L : g * L + w])
                return
            # tail engine plan for the last two units: g1 casts on scalar
            # with descs on sync, g0 casts on vector (free after the last
            # scan) with descs on scalar — two parallel two-step lanes
            for g in gorder:
                Y = Yt[g]
                Ysb = yout.tile([128, L], bf, name=f"Ysb{g}_{b}_{c}_{off}",
                                tag=f"Ysb{g}")
                dst = yT8s[b, c, 128 * g : 128 * (g + 1), off : off + w]
                ceng = nc.scalar if g == 1 else nc.vector
                oeng = nc.sync if g == 1 else nc.scalar
                (ceng.copy if ceng is nc.scalar else ceng.tensor_copy)(
                    Ysb[:, 0:w], Y[:, 0:w])
                oeng.dma_start(dst, Ysb[:, 0:w])

        # last chunk-unit split into two half-width sub-units: the final
        # scan then covers only 256 columns, so the tail chain after it
        # (2 matmuls + cast + store) moves half as much data
        units = [(c, b, 0, L) for c in range(NCHUNK) for b in range(BPC)]
        units = units[:-1] + [(NCHUNK - 1, BPC - 1, 0, L // 2),
                              (NCHUNK - 1, BPC - 1, L // 2, L // 2)]
        nu = len(units)
        st = []  # (unit args, U, zext, Yt)
        for k, (c, b, off, w) in enumerate(units):
            ti = nu - 1 - k
            ti = ti if ti <= 1 else None
            # WC(u_{k-2}) is emitted BEFORE V(u_k): V allocations wait on
            # the vps ring (tied to later scans) and would otherwise block
            # the already-ready WC work behind them in the in-order PE queue
            if k >= 2:
                s = st[k - 2]
                emit_wc(*s[0], s[1], s[2], s[3], tail_idx=s[4])
            U, zext = emit_vscan(c, b, off, w)
            st.append([(c, b, off, w), U, zext, None, ti])
            if k >= 1:
                s = st[k - 1]
                s[3] = emit_wu(*s[0], s[1], s[2], tail_idx=s[4])
        s = st[nu - 1]
        s[3] = emit_wu(*s[0], s[1], s[2], tail_idx=s[4])
        for k in (nu - 2, nu - 1):
            s = st[k]
            emit_wc(*s[0], s[1], s[2], s[3], tail_idx=s[4])

    _split_multi_waits(nc, mybir)
    return nc


def _host_prep(x0, u, Q, lam, Bmat, C, D):
    import ml_dtypes

    f = np.float32
    bfd = ml_dtypes.bfloat16
    lam = lam.astype(f)
    Bz = (Q.T.astype(f) @ Bmat.astype(f)).astype(f)      # (NX, NU)
    Cz = (C.astype(f) @ Q.astype(f)).astype(f)           # (NY, NX)
    z0 = (x0.astype(f) @ Q.astype(f)).astype(f)          # (B, NX)

    lam_p = np.stack([lam**j for j in range(MB)])         # (MB, NX)

    # W2[(i*32+u), n] = lam_n^(MB-1-i) * Bz[n, u]
    W2 = np.einsum("in,nu->iun", lam_p[::-1], Bz).reshape(MB * NU, NX)
    # WC[n, (32j+y)] = lam_n^j * Cz[y, n]
    WC = np.einsum("jn,yn->njy", lam_p, Cz).reshape(NX, MB * NY)
    # WU[(i*32+u), (32j+y)]
    WU = np.zeros((MB * NU, MB * NY), dtype=f)
    for j in range(MB):
        for i in range(MB):
            if i < j:
                Mji = (Cz * lam_p[j - 1 - i][None, :]) @ Bz   # (NY, NU)
                WU[i * NU : (i + 1) * NU, j * NY : (j + 1) * NY] = Mji.T
            elif i == j:
                WU[i * NU : (i + 1) * NU, j * NY : (j + 1) * NY] = D.T.astype(f)

    blocks = []
    for h in range(2):          # W2 order [g0h0, g1h0, g0h1, g1h1]
        for g in range(2):
            blocks.append(W2[128 * g : 128 * (g + 1), 128 * h : 128 * (h + 1)])
    for h in range(2):          # WC[h][g]
        for g in range(2):
            blocks.append(WC[128 * h : 128 * (h + 1), 128 * g : 128 * (g + 1)])
    # WU[g2][g] blocks; WU[1][0] is identically zero (i > j) and skipped
    blocks.append(WU[0:128, 0:128])      # WU00
    blocks.append(WU[0:128, 128:256])    # WU01
    blocks.append(WU[128:256, 128:256])  # WU11
    wAll = np.concatenate(blocks, axis=1).astype(bfd)     # (128, 11*128)
    wAll = np.asarray(wAll)

    # uT8[b, (i*32+u), k] = u[b, 8k+i, u]
    uT8 = np.ascontiguousarray(
        u.reshape(B, KCOL, MB, NU).transpose(0, 2, 3, 1).reshape(B, MB * NU, KCOL)
    ).astype(bfd)

    lam8 = (lam.astype(np.float64) ** MB).astype(f)
    lam8c = np.stack([lam8[:128], lam8[128:]], axis=1).astype(f)  # (128, 2)
    return wAll, z0, uT8, lam8c


def make_in_maps(x0, u, Q, lam, Bmat, C, D):
    f = np.float32
    wAll, z0, uT8, lam8c = _host_prep(x0, u, Q, lam, Bmat, C, D)
    in_maps = []
    for cidx in range(NCORES):
        sl = slice(cidx * BPC, (cidx + 1) * BPC)
        z0_c = z0[sl]
        # pzF col 2+2b+h = z0 of (batch b, state-half h)
        z0c = z0_c.reshape(BPC, 2, 128).transpose(2, 0, 1).reshape(128, 2 * BPC)
        pzF = np.zeros((128, 8), dtype=f)
        pzF[:, 0:2] = lam8c
        pzF[:, 2:6] = z0c
        ut = uT8[sl].reshape(BPC, 2, 128, KCOL)  # (b, g, row, k)
        # uH2[ch, g, row, :] = [b0 chunk-ch | b1 chunk-ch]
        uH2 = np.ascontiguousarray(
            np.stack(
                [
                    np.concatenate(
                        [ut[0, :, :, ch * L : (ch + 1) * L],
                         ut[1, :, :, ch * L : (ch + 1) * L]],
                        axis=2,
                    )
                    for ch in range(2)
                ],
                axis=0,
            )
        )
        in_maps.append(
            {
                "uH2": uH2,
                "pzF": pzF,
                "wAll": wAll,
            }
        )
    return in_maps


def kernel(x0, u, Q, lam, Bmat, C, D):
    global _PROG
    from concourse.bass_utils import run_bass_kernel_spmd

    if _PROG is None:
        _PROG = build_program()
    in_maps = make_in_maps(x0, u, Q, lam, Bmat, C, D)
    res = run_bass_kernel_spmd(_PROG, in_maps, list(range(NCORES)))
    y = np.empty((B, T, NY), dtype=np.float32)
    for cidx in range(NCORES):
        yT8s_c = res.results[cidx]["yT8s"].astype(np.float32)  # (BPC, 2, 256, L)
        # y[b, 8*(ch*L+k)+j, yy] = yT8s[b, ch, 32j+yy, k]
        y[cidx * BPC : (cidx + 1) * BPC] = (
            yT8s_c.reshape(BPC, 2, MB, NY, L)
            .transpose(0, 1, 4, 2, 3)
            .reshape(BPC, T, NY)
        )
    return y


# revision 48
# speedup vs baseline: 1.0102x; 1.0102x over previous
"""Diagonalizable linear plant (modal state-space scan) on 8 Trainium2 cores.

y[b,t] = Cz @ z[b,t-1] + D @ u[b,t],  z[b,t] = lam * z[b,t-1] + Bz @ u[b,t]
with z[b,-1] = z0[b] = x0[b] @ Q, Bz = Q^T Bmat, Cz = C Q.

Sharding: data-parallel over batch (16 batches -> 2 per core).

Block-8 formulation (the DVE scan instruction runs at ~2.45ns/column,
so the time axis is decimated 8x before it reaches the scan; everything
else is full 128x128xN=512 bf16 matmuls, fp32 PSUM):
  host packs u as uT8[(i*32+u), k] = u[8k+i, u]        (256 rows = 2 K-groups)
  PE   V_h = W2^T @ U          W2[(i,u),n] = lam_n^(7-i) Bz[n,u]
  DVE  zB = scan(lam^8, V)     block-boundary states z_{8k+7}
  PE   Y_g = WC^T @ zBprev + WU^T @ U     (g indexes (j,y) output groups)
       WC[n,(j,y)] = lam_n^j Cz[y,n]
       WU[(i,u),(j,y)] = (Cz lam^(j-1-i) Bz)[y,u] for i<j, D[y,u] for i=j, else 0
  host unpacks yT8[(32j+y), k] -> y[8k+j, y]

Scheduling notes (measured on HW, v17):
- DMA queues sustain ~110-220 GB/s each and co-drain everything queued
  on them, so transfers are grouped by need-time per queue and the late
  (ch1 / weight-tail) transfers are gated behind timed NOPs so the
  critical W2+ch0 set gets the full bandwidth first.
- ch0 moves as 2KB-row transfers (b0+b1 together): sub-2KB descriptor
  rows collapse queue throughput, and input bytes are conserved anyway.
- Vector runs only the 10 scans, back-to-back; lam^8/z0 ride a tiny f32
  param so no bf16 hi/lo recovery is needed.
- Y accumulators are one [128,1024] PSUM pair per unit (single cast),
  WU stages are emitted one unit behind V, WC two units behind, and the
  last chunk is split into two 256-col sub-units so only ~0.4us of
  matmul work trails the final scan.
- The final units' Y tiles live in the then-dead V-ring PSUM banks; the
  drain's sem-wait ladder is spread across all five engines and the
  final all-engine barrier is sequencer-only.
- The tile scheduler's DVE cost is repriced to the measured scan rate
  (~2.45ns/col) so its model matches the real critical path.
"""

import numpy as np

B, T, NX, NU, NY = 16, 8192, 256, 32, 32
NCORES = 8
BPC = B // NCORES   # batches per core
MB = 8              # time-block folded into matmul K
KCOL = T // MB      # block columns per batch (1024)
L = 512             # block-columns per chunk
NCHUNK = KCOL // L  # chunks per batch (2)
NWARM = 16          # PE warmup matmuls bridging the DMA fill window

_PROG = None  # built Bass program, cached across kernel() calls


def _patch_cost_model_scan():
    """The tile scheduler's cost model prices DVE ops at 1.04ns/elem, but
    the TensorTensorScan instruction measures ~2.45ns/column on hardware.
    The scans dominate this kernel's Vector load, and the mispricing makes
    the scheduler misplace the whole Y phase. Reprice DVE near the
    measured scan rate so the model's critical path matches hardware."""
    import concourse.hw_specs as hs
    import concourse.mybir as mybir

    if getattr(hs, "_dve_cycle_patched", False):
        return
    ct = dict(hs.TRN2Spec.CYCLE_T)
    ct[mybir.EngineType.DVE] = 1e9 / 0.5e9
    hs.TRN2Spec.CYCLE_T = ct
    hs._dve_cycle_patched = True


def _patch_sim_timed_nop():
    """The scheduling-pass CoreSim rejects the timed NOP (ISA opcode 164)
    used to gate late DMA enqueues. It has no simulator-visible side
    effects — treat it as a plain nop."""
    import concourse.bass_interp as bi

    if getattr(bi, "_timed_nop_patched", False):
        return
    orig = bi._visit_InstISA

    def _visit(isa_mod, instruction, core_sim):
        if instruction.isa_opcode == 164:  # NEURON_ISA_TPB_OPCODE_NOP
            return
        return orig(isa_mod, instruction, core_sim)

    bi._visit_InstISA = _visit
    bi._timed_nop_patched = True


def _patch_tile_drain():
    """walrus codegen in this container rejects >1 sync wait on one SP
    TPB_CTRL instruction (terminal TileContext drain / NoOp). Split the
    drain's waits across preceding SP nops carrying one wait each."""
    import concourse.tile as tile
    import concourse.mybir as mybir
    from concourse.vector_clock import ScopedClock

    if getattr(tile.TileContext, "_drain_patched", False):
        return

    def _drain_and_barrier(self, tick_clock, wait_clock):
        nc = self.nc
        scratch = nc.sync.nop()
        wait_clock.add_sem_waits(
            scratch.ins, ScopedClock({None: tick_clock.global_clock})
        )
        si = scratch.ins.sync_info
        waits = list(si.on_wait) if si is not None else []
        scratch.ins.sync_info = mybir.SyncInfo(on_wait=waits[:1], on_update=[])
        # spread the final sem waits across all engines: a serial ladder of
        # ~20 nops on sync alone costs >1us at the very end of the kernel
        engs = [nc.tensor, nc.vector, nc.scalar, nc.gpsimd, nc.sync]
        for k, w in enumerate(waits[1:]):
            n2 = engs[k % len(engs)].nop()
            n2.ins.sync_info = mybir.SyncInfo(on_wait=[w], on_update=[])
        nc.sync.drain()
        # the wait ladder above already proves every semaphore reached its
        # final value (all DMAs + engine work retired); a sequencer-level
        # barrier is enough to order the clear, and skips the per-engine
        # InstDrain ping-pong (~1us at the very end of the kernel)
        nc.all_engine_barrier(sem_only=True)
        assert self.sems is not None
        popped = nc._tile_sem_poison_stack.pop()
        assert popped is self._sem_poison
        nc.clear_and_free_semaphores(list(self.sems.allocated().values()))

    tile.TileContext._drain_and_barrier = _drain_and_barrier
    tile.TileContext._drain_patched = True


def _split_multi_waits(nc, mybir):
    """This container's walrus codegen accepts at most ONE sync wait per
    instruction. Hoist extra waits into standalone EventSemaphore nops on
    the same engine, placed immediately before the instruction."""
    ctr = [0]

    def fresh(engine, wait):
        ctr[0] += 1
        ev = mybir.InstEventSemaphore(name=f"I-wsplit-{ctr[0]}", ins=[], outs=[])
        ev.engine = engine
        ev.sync_info = mybir.SyncInfo(on_wait=[wait], on_update=[])
        nc.register_instruction(ev)
        return ev

    for fn in nc.m.functions:
        for bb in fn.blocks:
            out = []
            changed = False
            for inst in bb.instructions:
                si = inst.sync_info
                waits = list(si.on_wait) if si is not None else []
                if len(waits) > 1:
                    changed = True
                    for w in waits[:-1]:
                        out.append(fresh(inst.engine, w))
                    inst.sync_info = mybir.SyncInfo(
                        on_wait=[waits[-1]], on_update=list(si.on_update)
                    )
                out.append(inst)
            if changed:
                bb.instructions = out


def build_program():
    import concourse.bass as bass
    import concourse.tile as tile
    import concourse.mybir as mybir
    from contextlib import ExitStack

    _patch_cost_model_scan()
    _patch_sim_timed_nop()
    _patch_tile_drain()
    f32 = mybir.dt.float32
    bf = mybir.dt.bfloat16

    nc = bass.Bass()
    # uH2[ch, g, row, k]: 2 KB rows [b0 | b1] per chunk-half
    uH2 = nc.declare_dram_parameter("uH2", [2, 2, 128, 2 * L], bf, isOutput=False)
    # pzF: col 0:2 = lam^8 f32 (h0, h1); col 2+2b+h = z0 stream (b, h)
    pzF = nc.declare_dram_parameter("pzF", [128, 8], f32, isOutput=False)
    # wAll: W2 [g0h0,g1h0,g0h1,g1h1] | WC [h0g0,h0g1,h1g0,h1g1] | WU00,WU01,WU11
    wAll = nc.declare_dram_parameter("wAll", [128, 11 * 128], bf, isOutput=False)
    yT8s = nc.declare_dram_parameter("yT8s", [BPC, 2, 256, L], bf, isOutput=True)

    with ExitStack() as ctx:
        tc = ctx.enter_context(tile.TileContext(nc))
        const = ctx.enter_context(tc.tile_pool(name="const", bufs=1))
        vps = ctx.enter_context(tc.tile_pool(name="vps", bufs=2, space="PSUM"))
        yps = ctx.enter_context(tc.tile_pool(name="yps", bufs=2, space="PSUM"))
        zpool = ctx.enter_context(tc.tile_pool(name="z", bufs=6))
        yout = ctx.enter_context(tc.tile_pool(name="yo", bufs=4))

        # PE pstate warm-up: short dummy matmuls with no DMA deps keep the
        # Tensor engine continuously busy (the clock ramp needs ~3us of
        # uninterrupted execution to reach 2.4GHz) until real work lands.
        # Emitted FIRST: the tile scheduler keeps per-engine emission order,
        # so the dummy memset must precede everything else on vector.
        dummy = const.tile([128, 256], bf)
        nc.vector.memset(dummy[:], 0.0)
        WP = vps.tile([128, L], f32, name="WP", tag="V0")
        for _ in range(NWARM):
            nc.tensor.matmul(WP[:, 0:256], lhsT=dummy[:, 0:128],
                             rhs=dummy[:], start=True, stop=True)

        # --- DMA plan -----------------------------------------------------
        # Each queue sustains ~110-220 GB/s and throughput collapses for
        # descriptor rows under ~1KB, so: pzF (32B rows) rides the SWDGE
        # queue head where it only costs ~0.3us of lead time, W2 moves as a
        # single 1KB-row transfer, and the unit-0 critical set (W2 / U b0 g0
        # / U b0 g1) is spread across all THREE queues to land in parallel.
        # Transfers queued on the same DMA queue drain CONCURRENTLY (their
        # descriptors interleave), so a late-needed transfer steals
        # bandwidth from a critical one the moment its desc-gen runs. The
        # ch1 transfers are therefore gated behind timed nops on their
        # issuing engines so the ch0+W2 critical set gets the full
        # bandwidth of both HWDGE queues first.
        # sync  (HWDGE): U b0 g0, U b1 g1, U b1 g0, [gate] U ch1 g0;
        #                then g0 output stores and the tail g1 descs.
        # scalar(HWDGE): W2, U b0 g1, [gate] U ch1 g1; Y casts + g0 descs.
        # pool  (SWDGE): pzF, WU, WC; later the g1 output stores.
        pzt = const.tile([128, 8], f32)
        nc.gpsimd.dma_start(pzt[:], pzF[:, :])
        Wt = const.tile([128, 1408], bf)
        nc.scalar.dma_start(Wt[:, 0:512], wAll[:, 0:512])
        # UG[g]: [128, 2048] = [b0ch0 | b1ch0 | b0ch1 | b1ch1].
        # ch0 moves as one 2KB-row transfer per group (b0+b1 together):
        # descriptor rows under 2KB cost queue throughput, and the whole
        # input set is bandwidth-conserved anyway — landing b1 with b0
        # removes the scan3/4 arrival jitter entirely.
        UG = [const.tile([128, 4 * L], bf, name=f"UG{g}") for g in range(2)]
        nc.sync.dma_start(UG[0][:, 0 : 2 * L], uH2[0, 0, :, :])
        nc.scalar.dma_start(UG[1][:, 0 : 2 * L], uH2[0, 1, :, :])
        nc.gpsimd.nop(cycle_cnt=2800)
        nc.gpsimd.dma_start(Wt[:, 1024:1408], wAll[:, 1024:1408])
        nc.gpsimd.dma_start(Wt[:, 512:1024], wAll[:, 512:1024])
        nc.sync.nop(cycle_cnt=2800)
        nc.sync.dma_start(UG[0][:, 2 * L : 4 * L], uH2[1, 0, :, :])
        nc.scalar.nop(cycle_cnt=2000)
        nc.scalar.dma_start(UG[1][:, 2 * L : 4 * L], uH2[1, 1, :, :])

        lam8 = pzt[:, 0:2]
        z0t = pzt[:, 2:6]

        # lam^8 broadcast, built on DVE before any scan can start (vector is
        # otherwise idle during the DMA fill window; scans own it afterward)
        lam_bc = const.tile([128, 3 * L], f32)
        ones = lam_bc[:, 2 * L : 3 * L]
        nc.vector.memset(ones, 1.0)
        for h in range(2):
            nc.vector.tensor_scalar_mul(
                lam_bc[:, h * L : (h + 1) * L], ones, lam8[:, h : h + 1]
            )

        def wblk(i):
            return Wt[:, 128 * i : 128 * (i + 1)]

        W2 = [[wblk(0), wblk(2)], [wblk(1), wblk(3)]]          # [g][h]
        WC = [[wblk(4), wblk(5)], [wblk(6), wblk(7)]]          # [h][g]
        WU00 = wblk(8)
        WU01 = wblk(9)
        WU11 = wblk(10)                                        # WU[1][0] == 0

        mult = mybir.AluOpType.mult
        add = mybir.AluOpType.add

        # prev_z[b] = (Z tiles, end column) — the carry column for chaining
        prev_z = [[None, None] for _ in range(BPC)]
        prev_end = [0 for _ in range(BPC)]

        def emit_vscan(c, b, off, w):
            sl = slice((2 * c + b) * L + off, (2 * c + b) * L + off + w)
            U = [UG[0][:, sl], UG[1][:, sl]]
            zext = [None, None]
            for h in range(2):
                V = vps.tile([128, L], f32, name=f"V{h}_{b}_{c}_{off}",
                             tag=f"V{h}")
                Z = zpool.tile([128, L + 1], bf, name=f"Z{h}_{b}_{c}_{off}",
                               tag=f"Z{h}")
                carry = (z0t[:, 2 * b + h : 2 * b + h + 1]
                         if c == 0 and off == 0
                         else prev_z[b][h][:, prev_end[b] : prev_end[b] + 1])
                nc.tensor.matmul(V[:, 0:w], lhsT=W2[0][h], rhs=U[0],
                                 start=True, stop=False)
                nc.tensor.matmul(V[:, 0:w], lhsT=W2[1][h], rhs=U[1],
                                 start=False, stop=True)
                nc.vector.tensor_tensor_scan(
                    Z[:, 1 : w + 1], lam_bc[:, h * L : h * L + w], V[:, 0:w],
                    carry, mult, add,
                )
                nc.gpsimd.tensor_copy(Z[:, 0:1], carry)
                zext[h] = Z
            prev_z[b] = zext
            prev_end[b] = w
            return U, zext

        # The Y phase is emitted in two waves so the in-order PE queue can
        # frontload all scan-independent work: WU stages (inputs only) go
        # out one unit behind the V matmuls, WC stages (scan consumers)
        # two units behind. Only the last unit's two WC matmuls trail the
        # final scan.
        def emit_wu(c, b, off, w, U, zext, tail_idx=None):
            last = tail_idx == 0
            gorder = [1, 0] if last else [0, 1]
            Yt = {}
            if not last:
                # one [128, 2L] accumulator per unit: both groups cast in
                # ONE scalar op, halving the cast-throughput pressure
                Ypair = yps.tile([128, 2 * L], f32, name=f"Y_{b}_{c}_{off}",
                                 tag="Y")
                Yt["pair"] = Ypair
                for g in gorder:
                    Yt[g] = Ypair[:, g * L : (g + 1) * L]
            else:
                # the final unit's accumulators live in the V-ring banks
                # (dead once its scans retire): its matmuls never wait on
                # the cast-limited yps ring recycling
                for g in gorder:
                    Yt[g] = vps.tile([128, L], f32, name=f"Y{g}_{b}_{c}_{off}",
                                     tag=f"V{1 - g}")
            stages = [(1, WU01, U[0]), (0, WU00, U[0]), (1, WU11, U[1])]
            first = {0: True, 1: True}
            for g, lhsT, rhs in stages:
                nc.tensor.matmul(Yt[g][:, 0:w], lhsT=lhsT, rhs=rhs,
                                 start=first[g], stop=False)
                first[g] = False
            return Yt

        def emit_wc(c, b, off, w, U, zext, Yt, tail_idx=None):
            last = tail_idx == 0
            gorder = [1, 0] if (last or tail_idx == 1) else [0, 1]
            stages = [(gorder[0], 0), (gorder[1], 0), (gorder[0], 1),
                      (gorder[1], 1)]
            for g, h in stages:
                nc.tensor.matmul(Yt[g][:, 0:w], lhsT=WC[h][g],
                                 rhs=zext[h][:, 0:w],
                                 start=False, stop=(h == 1))
            if tail_idx is None:
                Ysb = yout.tile([128, 2 * L], bf, name=f"Ysb_{b}_{c}_{off}",
                                tag="Ysb")
                nc.scalar.copy(Ysb[:], Yt["pair"][:])
                for g in range(2):
                    dst = yT8s[b, c, 128 * g : 128 * (g + 1), off : off + w]
                    oeng = nc.sync if g == 0 else nc.gpsimd
                    oeng.dma_start(dst, Ysb[:, g * L : g * L + w])
                return
            # tail engine plan for the last two units: g1 casts on scalar
            # with descs on sync, g0 casts on vector (free after the last
            # scan) with descs on scalar — two parallel two-step lanes
            for g in gorder:
                Y = Yt[g]
                Ysb = yout.tile([128, L], bf, name=f"Ysb{g}_{b}_{c}_{off}",
                                tag=f"Ysb{g}")
                dst = yT8s[b, c, 128 * g : 128 * (g + 1), off : off + w]
                ceng = nc.scalar if g == 1 else nc.vector
                oeng = nc.sync if g == 1 else nc.scalar
                (ceng.copy if ceng is nc.scalar else ceng.tensor_copy)(
                    Ysb[:, 0:w], Y[:, 0:w])
                oeng.dma_start(dst, Ysb[:, 0:w])

        # last chunk-unit split into two half-width sub-units: the final
        # scan then covers only 256 columns, so the tail chain after it
        # (2 matmuls + cast + store) moves half as much data
        units = [(c, b, 0, L) for c in range(NCHUNK) for b in range(BPC)]
        units = units[:-1] + [(NCHUNK - 1, BPC - 1, 0, L // 2),
                              (NCHUNK - 1, BPC - 1, L // 2, L // 2)]
        nu = len(units)
        st = []  # (unit args, U, zext, Yt)
        for k, (c, b, off, w) in enumerate(units):
            ti = nu - 1 - k
            ti = ti if ti <= 1 else None
            # WC(u_{k-2}) is emitted BEFORE V(u_k): V allocations wait on
            # the vps ring (tied to later scans) and would otherwise block
            # the already-ready WC work behind them in the in-order PE queue
            if k >= 2:
                s = st[k - 2]
                emit_wc(*s[0], s[1], s[2], s[3], tail_idx=s[4])
            U, zext = emit_vscan(c, b, off, w)
            st.append([(c, b, off, w), U, zext, None, ti])
            if k >= 1:
                s = st[k - 1]
                s[3] = emit_wu(*s[0], s[1], s[2], tail_idx=s[4])
        s = st[nu - 1]
        s[3] = emit_wu(*s[0], s[1], s[2], tail_idx=s[4])
        for k in (nu - 2, nu - 1):
            s = st[k]
            emit_wc(*s[0], s[1], s[2], s[3], tail_idx=s[4])

    _split_multi_waits(nc, mybir)
    return nc


def _host_prep(x0, u, Q, lam, Bmat, C, D):
    import ml_dtypes

    f = np.float32
    bfd = ml_dtypes.bfloat16
    lam = lam.astype(f)
    Bz = (Q.T.astype(f) @ Bmat.astype(f)).astype(f)      # (NX, NU)
    Cz = (C.astype(f) @ Q.astype(f)).astype(f)           # (NY, NX)
    z0 = (x0.astype(f) @ Q.astype(f)).astype(f)          # (B, NX)

    lam_p = np.stack([lam**j for j in range(MB)])         # (MB, NX)

    # W2[(i*32+u), n] = lam_n^(MB-1-i) * Bz[n, u]
    W2 = np.einsum("in,nu->iun", lam_p[::-1], Bz).reshape(MB * NU, NX)
    # WC[n, (32j+y)] = lam_n^j * Cz[y, n]
    WC = np.einsum("jn,yn->njy", lam_p, Cz).reshape(NX, MB * NY)
    # WU[(i*32+u), (32j+y)]
    WU = np.zeros((MB * NU, MB * NY), dtype=f)
    for j in range(MB):
        for i in range(MB):
            if i < j:
                Mji = (Cz * lam_p[j - 1 - i][None, :]) @ Bz   # (NY, NU)
                WU[i * NU : (i + 1) * NU, j * NY : (j + 1) * NY] = Mji.T
            elif i == j:
                WU[i * NU : (i + 1) * NU, j * NY : (j + 1) * NY] = D.T.astype(f)

    blocks = []
    for h in range(2):          # W2 order [g0h0, g1h0, g0h1, g1h1]
        for g in range(2):
            blocks.append(W2[128 * g : 128 * (g + 1), 128 * h : 128 * (h + 1)])
    for h in range(2):          # WC[h][g]
        for g in range(2):
            blocks.append(WC[128 * h : 128 * (h + 1), 128 * g : 128 * (g + 1)])
    # WU[g2][g] blocks; WU[1][0] is identically zero (i > j) and skipped
    blocks.append(WU[0:128, 0:128])      # WU00
    blocks.append(WU[0:128, 128:256])    # WU01
    blocks.append(WU[128:256, 128:256])  # WU11
    wAll = np.concatenate(blocks, axis=1).astype(bfd)     # (128, 11*128)
    wAll = np.asarray(wAll)

    # uT8[b, (i*32+u), k] = u[b, 8k+i, u]
    uT8 = np.ascontiguousarray(
        u.reshape(B, KCOL, MB, NU).transpose(0, 2, 3, 1).reshape(B, MB * NU, KCOL)
    ).astype(bfd)

    lam8 = (lam.astype(np.float64) ** MB).astype(f)
    lam8c = np.stack([lam8[:128], lam8[128:]], axis=1).astype(f)  # (128, 2)
    return wAll, z0, uT8, lam8c


def make_in_maps(x0, u, Q, lam, Bmat, C, D):
    f = np.float32
    wAll, z0, uT8, lam8c = _host_prep(x0, u, Q, lam, Bmat, C, D)
    in_maps = []
    for cidx in range(NCORES):
        sl = slice(cidx * BPC, (cidx + 1) * BPC)
        z0_c = z0[sl]
        # pzF col 2+2b+h = z0 of (batch b, state-half h)
        z0c = z0_c.reshape(BPC, 2, 128).transpose(2, 0, 1).reshape(128, 2 * BPC)
        pzF = np.zeros((128, 8), dtype=f)
        pzF[:, 0:2] = lam8c
        pzF[:, 2:6] = z0c
        ut = uT8[sl].reshape(BPC, 2, 128, KCOL)  # (b, g, row, k)
        # uH2[ch, g, row, :] = [b0 chunk-ch | b1 chunk-ch]
        uH2 = np.ascontiguousarray(
            np.stack(
                [
                    np.concatenate(
                        [ut[0, :, :, ch * L : (ch + 1) * L],
                         ut[1, :, :, ch * L : (ch + 1) * L]],
                        axis=2,
                    )
                    for ch in range(2)
                ],
                axis=0,
            )
        )
        in_maps.append(
            {
                "uH2": uH2,
                "pzF": pzF,
                "wAll": wAll,
            }
        )
    return in_maps


def kernel(x0, u, Q, lam, Bmat, C, D):
    global _PROG
    from concourse.bass_utils import run_bass_kernel_spmd

    if _PROG is None:
        _PROG = build_program()
    in_maps = make_in_maps(x0, u, Q, lam, Bmat, C, D)
    res = run_bass_kernel_spmd(_PROG, in_maps, list(range(NCORES)))
    y = np.empty((B, T, NY), dtype=np.float32)
    for cidx in range(NCORES):
        yT8s_c = res.results[cidx]["yT8s"].astype(np.float32)  # (BPC, 2, 256, L)
        # y[b, 8*(ch*L+k)+j, yy] = yT8s[b, ch, 32j+yy, k]
        y[cidx * BPC : (cidx + 1) * BPC] = (
            yT8s_c.reshape(BPC, 2, MB, NY, L)
            .transpose(0, 1, 4, 2, 3)
            .reshape(BPC, T, NY)
        )
    return y
